# revision 36
# baseline (speedup 1.0000x reference)
"""Trainium2 Bass kernel for mixed Gaussian/Gabor splat rasterization.

Problem: render 3072 plain 2D gaussians + 1024 gabor-modulated gaussians
(G=4 cosine carriers each) densely into a [1,3,256,256] image, clamp to [0,1].

The axon-tunneled PJRT dispatch dominates wall time (~85 MB/s effective
host->device bandwidth, large per-array overhead), so the design minimizes
per-call transferred bytes rather than device FLOPs:

  - Pixels sharded across 8 cores: core k owns image rows [32k, 32k+32),
    processed as 8 column-blocks of 32x32 px in a block-centered frame.
  - Host precomputes per-gaussian sigma planes w0..w5 (sigma(px) = w0 dx^2 +
    w1 dxdy + w2 dy^2 + w3 dx + w4 dy + w5 in centered pixel coords) in f64,
    and selects, per core, only the gaussians whose exp(-sigma) can exceed
    e^-14 anywhere in that core's row stripe (y-extent test via the
    x-minimized quadratic).  ~500 low + ~300 high per core instead of 4096.
  - Everything the device needs ships in TWO packed arrays per core
    (one f32, one bf16) + a uint8 output (x255), ~70 KB/core total vs
    ~960 KB for the dense/full-precision layout.
  - The pixel-basis tensors for the sigma / gabor-phase matmuls (identical
    for every block and core) are generated on device with iota +
    elementwise ops; the 128x128 transpose identity comes from affine_select.
  - Compute structure per block (as in the dense version): K=13 split-f32r
    matmul for sigma (bf16 hi/lo weight split sidesteps f32r's ~11-bit
    mantissa), Exp with the per-gaussian constant riding the ACT bias,
    gabor carriers via half-angle Sin + diag(-2wg) PSUM matmuls, image
    accumulation img[3,px] += colors^T @ W chained over chunks in PSUM.
"""

import math
import numpy as np

try:
    import concourse.bass as bass
except ImportError:
    import sys
    sys.path.insert(0, "/opt/trn_rl_repo")
    import concourse.bass as bass

import ml_dtypes
import concourse.tile as tile
from concourse import bacc, mybir
from concourse.bass_utils import run_bass_kernel_spmd

F32 = mybir.dt.float32
F32R = mybir.dt.float32r
BF16 = mybir.dt.bfloat16
I32 = mybir.dt.int32
U8 = mybir.dt.uint8
OP = mybir.AluOpType
AF = mybir.ActivationFunctionType

H = 256
W = 256
NL = 3072
NH = 1024
G = 4
NCORES = 8
ROWS = H // NCORES          # 32 rows per core
PX = ROWS * W               # 8192 pixels per core
SB = 1024                   # superblock = 32 cols x 32 rows
NSB = PX // SB              # 8 column blocks
CB = 32                     # columns per superblock
INV2PI = 1.0 / (2.0 * math.pi)
TCUT = 14.0                 # exp(-14) ~ 8e-7: y-extent cutoff
MAGIC = 1.5 * 2 ** 23       # round-to-int via add/sub
BF16NP = ml_dtypes.bfloat16

_CACHE = {}


def _x0(sb):
    # x-center of column block sb (in centered image coords)
    return 32.0 * sb - 112.0


def _build_program(nlc, nhc, actL, actH, two_pass=True, repeat=1):
    nc = bacc.Bacc("TRN2", target_bir_lowering=False, debug=False)
    F = nlc * 6 + nhc * 16 + 1
    Bc = (nlc + nhc) * 3 + nhc * G
    pf = nc.declare_dram_parameter("pf", [128, F], F32, isOutput=False)
    pb = nc.declare_dram_parameter("pb", [128, Bc], BF16, isOutput=False)
    out_ext = nc.declare_dram_parameter("out", [3, PX], U8, isOutput=True)

    with tile.TileContext(nc, pool_alloc_mode="queue") as tc:
        with tc.tile_pool(name="singles", bufs=1) as singles:
            _body(nc, tc, singles, pf, pb, out_ext, nlc, nhc, actL, actH,
                  two_pass, repeat)
    nc.finalize()
    return nc


def _body(nc, tc, singles, pf, pb, out_ext, nlc, nhc, actL, actH, two_pass,
          repeat=1):
    V = nc.vector
    S = nc.scalar
    T = nc.tensor
    nch = nlc + nhc
    F = nlc * 6 + nhc * 16 + 1
    Bc = nch * 3 + nhc * G

    # ---------------- packed inputs ----------------
    pf_sb = singles.tile([128, F], F32)
    nc.gpsimd.dma_start(out=pf_sb, in_=pf[:])
    pb_sb = singles.tile([128, Bc], BF16)
    nc.gpsimd.dma_start(out=pb_sb, in_=pb[:])

    w6L = pf_sb[:, 0:nlc * 6].rearrange("p (c k) -> p c k", k=6)
    oH = nlc * 6
    w6H = pf_sb[:, oH:oH + nhc * 8].rearrange("p (c k) -> p c k", k=8)
    oG = oH + nhc * 8
    gab = pf_sb[:, oG:oG + nhc * 8].rearrange("p (c k) -> p c k", k=8)
    ycen_sb = pf_sb[:, F - 1:F]
    c3 = pb_sb[:, 0:nch * 3].rearrange("p (c k) -> p c k", k=3)
    wgv = pb_sb[:, nch * 3:nch * 3 + nhc * G].rearrange("p (c g) -> p c g", g=G)

    # per-(block, chunk) slot maps for the block-active chunk lists
    modslot = {}
    for b in range(NSB):
        for hc in actH[b]:
            modslot[(b, hc)] = len(modslot)
    n_modslots = max(1, len(modslot))
    maxactH = max(1, max(len(a) for a in actH))
    maxact = max(1, max(len(actL[b]) + len(actH[b]) for b in range(NSB)))

    # ---------------- persistent SBUF tensors ----------------
    ident_sb = singles.tile([128, 128], F32)
    basis_sb = singles.tile([13, SB], F32R)
    basisq_sb = singles.tile([6, SB], F32R)
    modsb = singles.tile([128, n_modslots, SB], BF16)
    diag = singles.tile([128, nhc * G * 128], BF16)
    swg = singles.tile([128, nhc], F32)
    fsl0 = singles.tile([128, nhc, G], F32)
    fsl1 = singles.tile([128, nhc, G], F32)
    r0 = singles.tile([128, nhc, G], F32)
    r1 = singles.tile([128, nhc, G], F32)
    f2gy = singles.tile([128, nhc, G], F32)
    fpl = singles.tile([128, nhc, G, 8], F32)
    wqL = singles.tile([128, nlc, 16], F32)
    wqH = singles.tile([128, nhc, 16], F32)
    w3yL = singles.tile([128, nlc], F32)
    w4yL = singles.tile([128, nlc], F32)
    n5yL = singles.tile([128, nlc], F32)
    w3yH = singles.tile([128, nhc], F32)
    w4yH = singles.tile([128, nhc], F32)
    n5yH = singles.tile([128, nhc], F32)

    # ---------------- launch-once prep ----------------
    with tc.tile_pool(name="prep", bufs=1) as prep:
        # identity for transposes: diag(p==j) via affine_select on ones
        ones = prep.tile([128, 128], F32)
        V.tensor_scalar(ones, pf_sb[:, 0:1].to_broadcast([128, 128]),
                        0.0, 1.0, OP.mult, OP.add)
        nc.gpsimd.affine_select(out=ident_sb, in_=ones, pattern=[[-1, 128]],
                                base=0, channel_multiplier=1,
                                compare_op=OP.is_equal, fill=0.0)

        # pixel-basis rows, identical for every block/core, computed on
        # partition 0 (engine APs can't start at arbitrary partitions) and
        # DMA'd to their target rows.
        # block is 32x32, y-major; xc = (px % 32) - 15.5, yc = (px//32) - 15.5
        xi = prep.tile([1, SB], I32)
        nc.gpsimd.iota(xi, pattern=[[0, 32], [1, 32]], base=0,
                       channel_multiplier=0)
        yi = prep.tile([1, SB], I32)
        nc.gpsimd.iota(yi, pattern=[[1, 32], [0, 32]], base=0,
                       channel_multiplier=0)
        xc = prep.tile([1, SB], F32)
        V.tensor_copy(out=xc, in_=xi)
        V.tensor_scalar(xc, xc, -15.5, None, OP.add)
        yc = prep.tile([1, SB], F32)
        V.tensor_copy(out=yc, in_=yi)
        V.tensor_scalar(yc, yc, -15.5, None, OP.add)

        # ---- pass-A dependencies first ----
        # gabor-phase basis rows: [xc, yc, q0..q3] with one-hot 16x16
        # quarter indicators, q = 2*(xc>=0) + (yc>=0)
        bxc = prep.tile([1, SB], F32R, name="bxc")
        V.tensor_copy(out=bxc, in_=xc)
        byc = prep.tile([1, SB], F32R, name="byc")
        V.tensor_copy(out=byc, in_=yc)
        xge = prep.tile([1, SB], F32)
        V.tensor_scalar(xge, xc, 0.0, None, OP.is_ge)
        yge = prep.tile([1, SB], F32)
        V.tensor_scalar(yge, yc, 0.0, None, OP.is_ge)
        q3 = prep.tile([1, SB], F32R)
        V.tensor_tensor(out=q3, in0=xge, in1=yge, op=OP.mult)
        q2 = prep.tile([1, SB], F32R)
        V.tensor_tensor(out=q2, in0=xge, in1=q3, op=OP.subtract)
        q1 = prep.tile([1, SB], F32R)
        V.tensor_tensor(out=q1, in0=yge, in1=q3, op=OP.subtract)
        q0 = prep.tile([1, SB], F32R)
        V.tensor_tensor(out=q0, in0=xge, in1=yge, op=OP.add)
        V.tensor_scalar(q0, q0, -1.0, 1.0, OP.mult, OP.add)
        V.tensor_tensor(out=q0, in0=q0, in1=q3, op=OP.add)
        for j, src in enumerate([bxc, byc, q0, q1, q2, q3]):
            nc.gpsimd.dma_start(out=basisq_sb[j:j + 1, :], in_=src)

        # gabor: phase slopes (cycles/px), y-folded constants, carrier sums
        V.tensor_scalar(fsl0, gab[:, :, 0:G], INV2PI, None, OP.mult)
        V.tensor_scalar(fsl1, gab[:, :, G:2 * G], INV2PI, None, OP.mult)
        V.tensor_scalar(r0, fsl0, 8.0, None, OP.mult)
        V.tensor_scalar(r1, fsl1, 8.0, None, OP.mult)
        pa = prep.tile([128, nhc], F32)
        pc = prep.tile([128, nhc], F32)
        for g in range(G):
            V.tensor_tensor(out=pa, in0=gab[:, :, g], in1=w6H[:, :, 6],
                            op=OP.mult)
            V.tensor_tensor(out=pc, in0=gab[:, :, G + g], in1=w6H[:, :, 7],
                            op=OP.mult)
            V.tensor_tensor(out=pa, in0=pa, in1=pc, op=OP.add)
            V.tensor_scalar(pa, pa, -INV2PI, None, OP.mult)
            V.scalar_tensor_tensor(out=f2gy[:, :, g], in0=fsl1[:, :, g],
                                   scalar=ycen_sb, in1=pa,
                                   op0=OP.mult, op1=OP.add)
        V.tensor_copy(out=fpl[:, :, :, 0], in_=fsl0)
        V.tensor_copy(out=fpl[:, :, :, 1], in_=fsl1)

        V.tensor_tensor(out=swg, in0=wgv[:, :, 0], in1=wgv[:, :, 1], op=OP.add)
        V.tensor_tensor(out=swg, in0=swg, in1=wgv[:, :, 2], op=OP.add)
        V.tensor_tensor(out=swg, in0=swg, in1=wgv[:, :, 3], op=OP.add)
        wgm2 = prep.tile([128, nhc, G], F32)
        V.tensor_scalar(wgm2, wgv, -2.0, None, OP.mult)
        for c in range(nhc):
            for g in range(G):
                V.tensor_tensor(
                    out=diag[:, (c * G + g) * 128:(c * G + g + 1) * 128],
                    in0=ident_sb,
                    in1=wgm2[:, c, g:g + 1].to_broadcast([128, 128]),
                    op=OP.mult)

        # ---- pass-B dependencies (overlap with pass A) ----
        # quadratic products + bf16 hi/lo splits (f32r staging tiles)
        def _split(a, b, nm, E):
            pr = prep.tile([1, SB], F32, name=f"pr{nm}")
            E.tensor_tensor(out=pr, in0=a, in1=b, op=OP.mult)
            h16 = prep.tile([1, SB], BF16, name=f"h16{nm}")
            E.tensor_copy(out=h16, in_=pr)
            hi = prep.tile([1, SB], F32R, name=f"hi{nm}")
            E.tensor_copy(out=hi, in_=h16)
            lo = prep.tile([1, SB], F32R, name=f"lo{nm}")
            E.tensor_tensor(out=lo, in0=pr, in1=h16, op=OP.subtract)
            return hi, lo
        bxh, bxl = _split(xc, xc, "xx", V)
        bxyh, bxyl = _split(xc, yc, "xy", V)
        byh, byl = _split(yc, yc, "yy", V)
        # rows: [x2h,x2l,x2h, xyh,xyl,xyh, y2h,y2l,y2h, xc,xc, yc,yc]
        srcs = [bxh, bxl, bxh, bxyh, bxyl, bxyh, byh, byl, byh,
                bxc, bxc, byc, byc]
        for j, src in enumerate(srcs):
            nc.gpsimd.dma_start(out=basis_sb[j:j + 1, :], in_=src)

        # per-launch derived per-gaussian tensors
        ycen2 = prep.tile([128, 1], F32)
        V.tensor_tensor(out=ycen2, in0=ycen_sb, in1=ycen_sb, op=OP.mult)
        ycen_2x = prep.tile([128, 1], F32)
        V.tensor_scalar(ycen_2x, ycen_sb, 2.0, None, OP.mult)

        for key, n, w6, wq, w3y, w4y, n5y in (
                ("L", nlc, w6L, wqL, w3yL, w4yL, n5yL),
                ("H", nhc, w6H, wqH, w3yH, w4yH, n5yH)):
            # bf16 hi/lo split of the (block-independent) quadratic weights
            # into wq rows 0..8: [w0h,w0h,w0l, w1h,w1h,w1l, w2h,w2h,w2l]
            for j in range(3):
                hi = prep.tile([128, n, 1], BF16, name=f"hi{key}{j}")
                V.tensor_copy(out=hi, in_=w6[:, :, j:j + 1])
                V.tensor_copy(
                    out=wq[:, :, 3 * j:3 * j + 2],
                    in_=hi.to_broadcast([128, n, 2]))
                V.tensor_tensor(out=wq[:, :, 3 * j + 2], in0=w6[:, :, j],
                                in1=hi[:, :, 0], op=OP.subtract)
            # y-recentered linear/const planes (y0 is fixed per core):
            # w3y = w3 + y0 w1 ; w4y = w4 + 2 y0 w2
            # n5y = -(w5 + y0 w4 + y0^2 w2)
            V.scalar_tensor_tensor(out=w3y, in0=w6[:, :, 1], scalar=ycen_sb,
                                   in1=w6[:, :, 3], op0=OP.mult, op1=OP.add)
            V.scalar_tensor_tensor(out=w4y, in0=w6[:, :, 2], scalar=ycen_2x,
                                   in1=w6[:, :, 4], op0=OP.mult, op1=OP.add)
            V.scalar_tensor_tensor(out=n5y, in0=w6[:, :, 4], scalar=ycen_sb,
                                   in1=w6[:, :, 5], op0=OP.mult, op1=OP.add)
            V.scalar_tensor_tensor(out=n5y, in0=w6[:, :, 2], scalar=ycen2,
                                   in1=n5y, op0=OP.mult, op1=OP.add)
            V.tensor_scalar(n5y, n5y, -1.0, None, OP.mult)

    # ---------------- main loops over column blocks ----------------
    # two_pass=True runs the Sin pass for all 8 blocks, then the Exp pass,
    # so the ACT engine loads each activation-table set once instead of
    # per block; modsb buffers all 8 blocks' modulation.
    with tc.tile_pool(name="quad", bufs=2, space="PSUM") as quad, \
         tc.tile_pool(name="mi", bufs=2, space="PSUM") as mi, \
         tc.tile_pool(name="wrk", bufs=3) as wrk, \
         tc.tile_pool(name="spool", bufs=2) as spool, \
         tc.tile_pool(name="s2pool", bufs=2) as s2pool, \
         tc.tile_pool(name="sbw", bufs=2) as sbw, \
         tc.tile_pool(name="outp", bufs=2) as outp:

        # ---- pass A: SIN (half-angle: cos(2pi t) = 1 - 2 sin^2(pi t)) ----
        def pass_a(sb):
            bs = sb * SB
            x0 = _x0(sb)
            # gabor-phase planes rows 2..5: per-quarter rounded offsets
            # fq = fbb - round(fbb + sx*8*f0 + sy*8*f1), quarter q=2*xh+yh
            fbb = sbw.tile([128, nhc, G], F32, name="fbb", tag="fbb")
            V.scalar_tensor_tensor(out=fbb, in0=fsl0, scalar=x0, in1=f2gy,
                                   op0=OP.mult, op1=OP.add)
            fbt = sbw.tile([128, nhc, G], F32, name="fbt", tag="fbt")
            for q in range(4):
                V.tensor_tensor(out=fbt, in0=fbb, in1=r0,
                                op=(OP.add if q >= 2 else OP.subtract))
                V.tensor_tensor(out=fbt, in0=fbt, in1=r1,
                                op=(OP.add if q % 2 else OP.subtract))
                V.tensor_scalar(fbt, fbt, MAGIC, MAGIC, OP.add, OP.subtract)
                V.tensor_tensor(out=fpl[:, :, :, 2 + q], in0=fbb, in1=fbt,
                                op=OP.subtract)
            # transpose only the block-active high chunks to lhsT layout
            fT = sbw.tile([6, maxactH * G * 128], F32R, name="fT", tag="fT")
            for j, hc in enumerate(actH[sb]):
                tpF = quad.tile([6, G * 128], F32, name="tpF", tag="quad")
                for g in range(G):
                    T.transpose(tpF[:, g * 128:(g + 1) * 128],
                                fpl[:, hc, g, 0:6], ident_sb)
                V.tensor_copy(out=fT[:, j * G * 128:(j + 1) * G * 128],
                              in_=tpF)

            for j, hc in enumerate(actH[sb]):
                mod_ps = mi.tile([128, SB], F32, name="mod_ps", tag="mi")
                for g in range(G):
                    t_ps = quad.tile([128, SB], F32, name="t_ps", tag="quad")
                    for h in range(2):
                        T.matmul(
                            t_ps[:, h * 512:(h + 1) * 512],
                            fT[:, (j * G + g) * 128:(j * G + g + 1) * 128],
                            basisq_sb[:, h * 512:(h + 1) * 512],
                            start=True, stop=True)
                    sg = spool.tile([128, SB], F32, name="sg")
                    S.activation(sg, t_ps, AF.Sin, scale=math.pi)
                    s2 = s2pool.tile([128, SB], BF16, name="s2")
                    nc.gpsimd.tensor_tensor(out=s2, in0=sg, in1=sg,
                                            op=OP.mult)
                    for h in range(2):
                        T.matmul(
                            mod_ps[:, h * 512:(h + 1) * 512],
                            diag[:, (hc * G + g) * 128:(hc * G + g + 1) * 128],
                            s2[:, h * 512:(h + 1) * 512],
                            start=(g == 0), stop=(g == G - 1))
                V.tensor_copy(out=modsb[:, modslot[(sb, hc)], :],
                              in_=mod_ps)

        # ---- pass B: EXP envelopes + image accumulation ----
        def pass_b(sb):
            bs = sb * SB
            x0 = _x0(sb)
            # per-block recentered linear/const sigma planes:
            # w3' = w3y + 2 x0 w0 ; w4' = w4y + x0 w1
            # nw5 = n5y - x0 w3y - x0^2 w0
            n5 = {}
            for key, n, w6, wq, w3y, w4y, n5y, E in (
                    ("L", nlc, w6L, wqL, w3yL, w4yL, n5yL, V),
                    ("H", nhc, w6H, wqH, w3yH, w4yH, n5yH, V)):
                wp3 = sbw.tile([128, n], F32, name=f"wp3{key}", tag=f"w3{key}")
                E.scalar_tensor_tensor(out=wp3, in0=w6[:, :, 0],
                                       scalar=2.0 * x0, in1=w3y,
                                       op0=OP.mult, op1=OP.add)
                wp4 = sbw.tile([128, n], F32, name=f"wp4{key}", tag=f"w4{key}")
                E.scalar_tensor_tensor(out=wp4, in0=w6[:, :, 1], scalar=x0,
                                       in1=w4y, op0=OP.mult, op1=OP.add)
                n5t = sbw.tile([128, n], F32, name=f"n5{key}", tag=f"n5{key}")
                E.scalar_tensor_tensor(out=n5t, in0=w3y, scalar=-x0,
                                       in1=n5y, op0=OP.mult, op1=OP.add)
                E.scalar_tensor_tensor(out=n5t, in0=w6[:, :, 0],
                                       scalar=-x0 * x0, in1=n5t,
                                       op0=OP.mult, op1=OP.add)
                n5[key] = n5t
                # wq rows 9..12: bf16 hi/lo of w3', w4'
                for src, base in ((wp3, 9), (wp4, 11)):
                    hh = sbw.tile([128, n], BF16, name=f"hh{key}{base}",
                                  tag=f"hh{key}{base}")
                    E.tensor_copy(out=hh, in_=src)
                    E.tensor_copy(out=wq[:, :, base], in_=hh)
                    E.tensor_tensor(out=wq[:, :, base + 1], in0=src,
                                    in1=hh, op=OP.subtract)

            # block-active chunk list: (global c3 index, group key, local idx)
            chunks = ([(c, "L", c) for c in actL[sb]]
                      + [(nlc + c, "H", c) for c in actH[sb]])

            # transpose only active weights -> g5t
            g5t = sbw.tile([13, maxact * 128], F32R, name="g5t", tag="g5t")
            for q in range((len(chunks) + 7) // 8):
                cs = q * 8
                ce = min(cs + 8, len(chunks))
                tp5 = quad.tile([13, 1024], F32, name="tp5", tag="quad")
                for j in range(cs, ce):
                    _, key, cl = chunks[j]
                    wq = wqL if key == "L" else wqH
                    T.transpose(tp5[:, (j - cs) * 128:(j - cs + 1) * 128],
                                wq[:, cl, 0:13], ident_sb)
                V.tensor_copy(out=g5t[:, cs * 128:ce * 128],
                              in_=tp5[:, 0:(ce - cs) * 128])

            img_ps = mi.tile([3, SB], F32, name="img_ps", tag="mi")
            for j, (cg, key, cl) in enumerate(chunks):
                sig_ps = quad.tile([128, SB], F32, name="sig_ps", tag="quad")
                for h in range(2):
                    T.matmul(
                        sig_ps[:, h * 512:(h + 1) * 512],
                        g5t[:, j * 128:(j + 1) * 128],
                        basis_sb[:, h * 512:(h + 1) * 512],
                        start=True, stop=True)
                w = wrk.tile([128, SB], BF16, name="w", tag="w")
                if key == "L":
                    S.activation(w, sig_ps, AF.Exp,
                                 bias=n5["L"][:, cl:cl + 1], scale=-1.0)
                else:
                    env = wrk.tile([128, SB], BF16, name="env", tag="env")
                    S.activation(env, sig_ps, AF.Exp,
                                 bias=n5["H"][:, cl:cl + 1], scale=-1.0)
                    V.scalar_tensor_tensor(
                        out=w, in0=modsb[:, modslot[(sb, cl)], :],
                        scalar=swg[:, cl:cl + 1], in1=env,
                        op0=OP.add, op1=OP.mult)
                for h in range(2):
                    T.matmul(
                        img_ps[:, h * 512:(h + 1) * 512],
                        c3[:, cg, :],
                        w[:, h * 512:(h + 1) * 512],
                        start=(j == 0), stop=(j == len(chunks) - 1))

            outf = outp.tile([3, SB], F32, name="outf", tag="outf")
            V.tensor_scalar(outf, img_ps, 0.0, 1.0, OP.max, OP.min)
            # u8 convert rounds to nearest on HW; no +0.5 pre-bias
            V.tensor_scalar(outf, outf, 255.0, None, OP.mult)
            outu = outp.tile([3, SB], U8, name="outu", tag="outu")
            V.tensor_copy(out=outu, in_=outf)
            nc.gpsimd.dma_start(out=out_ext[:, bs:bs + SB], in_=outu)

        for _rep in range(repeat):
            if two_pass:
                for sb in range(NSB):
                    pass_a(sb)
                for sb in range(NSB):
                    pass_b(sb)
            else:
                for sb in range(NSB):
                    pass_a(sb)
                    pass_b(sb)


def _planes(mu, chol, feat, opac, gfx=None, gfy=None):
    """Per-gaussian sigma planes + color, f64 internally."""
    mu = np.asarray(mu, np.float64)
    chol = np.asarray(chol, np.float64)
    m = np.tanh(mu)
    xci = m[:, 0] * 128.0
    yci = m[:, 1] * 128.0
    l1 = chol[:, 0] + 0.5
    l2 = chol[:, 1]
    l3 = chol[:, 2] + 0.5
    sxx = l1 * l1
    sxy = l1 * l2
    syy = l2 * l2 + l3 * l3
    det = sxx * syy - sxy * sxy
    A = syy / det
    Bq = -sxy / det
    C = sxx / det
    w6 = np.stack([0.5 * A, Bq, 0.5 * C,
                   -(A * xci + Bq * yci), -(Bq * xci + C * yci),
                   0.5 * (A * xci * xci + 2 * Bq * xci * yci
                          + C * yci * yci)], axis=1)
    colf = np.asarray(feat, np.float64) * np.asarray(opac, np.float64)
    ayy = np.maximum(C - Bq * Bq / np.maximum(A, 1e-30), 1e-12)
    ry = np.sqrt(2.0 * TCUT / ayy)
    yabs = yci + 128.0
    return w6, colf, xci, yci, yabs, ry


def _gather(arr, idx, cap, fill=0.0):
    out = np.full((cap,) + arr.shape[1:], fill, np.float64)
    out[:len(idx)] = arr[idx]
    return out


def _chunk128(a, nch):
    # [nch*128, k] -> [128, nch, k] with gaussian j -> (chunk j//128, part j%128)
    return np.ascontiguousarray(
        a.reshape(nch, 128, -1).transpose(1, 0, 2))


def _rx_of(w6):
    # x-extent radius at the exp(-TCUT) cutoff (x-minimized quadratic)
    A = 2 * w6[:, 0]
    Bq = w6[:, 1]
    C = 2 * w6[:, 2]
    axx = np.maximum(A - Bq * Bq / np.maximum(C, 1e-30), 1e-12)
    return np.sqrt(2.0 * TCUT / axx)


def _bucketize(xa, ya, ry, rx, n0):
    """Per-core y-stripe selection + x-bucket assignment with <=128 per
    bucket (n buckets, shared across cores).  Returns n, per-core
    (stripe-index, bucket-id) pairs, and per-(block, bucket) active flags."""
    for n in range(max(1, n0), 65):
        bw = W / n
        sel = []
        ok = True
        for k in range(NCORES):
            ylo, yhi = ROWS * k, ROWS * (k + 1)
            idx = np.nonzero((ya + ry >= ylo) & (ya - ry <= yhi))[0]
            bi = np.clip((xa[idx] / bw).astype(np.int64), 0, n - 1)
            if len(idx) and np.bincount(bi, minlength=n).max() > 128:
                ok = False
                break
            sel.append((idx, bi))
        if ok:
            active = np.zeros((NSB, n), bool)
            for idx, bi in sel:
                for b in range(NSB):
                    xlo, xhi = CB * b, CB * (b + 1)
                    m = (xa[idx] + rx[idx] >= xlo) & (xa[idx] - rx[idx] <= xhi)
                    active[b][np.unique(bi[m])] = True
            return n, sel, active
    raise RuntimeError("bucket overflow")


def _host_inputs(low_mu, high_mu, low_chol, high_chol, low_feat, high_feat,
                 low_opac, high_opac, gabor_freqs, gabor_weights):
    w6L, colL, xciL, _, yaL, ryL = _planes(low_mu, low_chol, low_feat,
                                           low_opac)
    w6H, colH, xciH, yciH, yaH, ryH = _planes(high_mu, high_chol, high_feat,
                                              high_opac)
    gf = np.asarray(gabor_freqs, np.float64)
    gw = np.asarray(gabor_weights, np.float64)
    fx = gf[:, 0].reshape(-1, G)
    fy = gf[:, 1].reshape(-1, G)
    wg = gw[:, 0].reshape(-1, G)
    xaL = xciL + 128.0
    xaH = xciH + 128.0

    nlc, selL, actLf = _bucketize(xaL, yaL, ryL, _rx_of(w6L), 1)
    nhc, selH, actHf = _bucketize(xaH, yaH, ryH, _rx_of(w6H), 1)
    # block-active chunk lists (guarantee >=1 chunk so img_ps gets written)
    actL = tuple(tuple(int(c) for c in np.nonzero(actLf[b])[0]) or (0,)
                 for b in range(NSB))
    actH = tuple(tuple(int(c) for c in np.nonzero(actHf[b])[0])
                 for b in range(NSB))

    in_maps = []
    for k in range(NCORES):
        idxL, biL = selL[k]
        idxH, biH = selH[k]
        ordL = [idxL[biL == c] for c in range(nlc)]
        ordH = [idxH[biH == c] for c in range(nhc)]

        def pack(arr, order, fill=0.0):
            return np.concatenate(
                [_gather(arr, o, 128, fill) for o in order], axis=0)

        w6l = pack(w6L, ordL)
        w6h = pack(w6H, ordH)
        for a, order in ((w6l, ordL), (w6h, ordH)):
            for c, o in enumerate(order):
                a[c * 128 + len(o):(c + 1) * 128, 5] = 1000.0  # pad: exp->0
        xy_h = np.stack([pack(xciH, ordH), pack(yciH, ordH)], axis=1)
        gabk = np.concatenate([pack(fx, ordH), pack(fy, ordH)], axis=1)
        y0 = np.full((128, 1), 32.0 * k - 112.0)
        pf32 = np.concatenate([
            _chunk128(w6l, nlc).reshape(128, nlc * 6),
            _chunk128(np.concatenate([w6h, xy_h], axis=1),
                      nhc).reshape(128, nhc * 8),
            _chunk128(gabk, nhc).reshape(128, nhc * 8),
            y0], axis=1).astype(np.float32)
        c3k = np.concatenate([_chunk128(pack(colL, ordL),
                                        nlc).reshape(128, nlc * 3),
                              _chunk128(pack(colH, ordH),
                                        nhc).reshape(128, nhc * 3)], axis=1)
        wgk = _chunk128(pack(wg, ordH), nhc).reshape(128, nhc * G)
        pbf = np.concatenate([c3k, wgk], axis=1).astype(np.float32)
        in_maps.append({
            "pf": np.ascontiguousarray(pf32),
            "pb": np.ascontiguousarray(pbf.astype(BF16NP)),
        })
    return in_maps, nlc, nhc, actL, actH


def _assemble(results):
    """Reassemble per-core column-block uint8 outputs into [1,3,256,256]."""
    img = np.zeros((3, H, W), np.float32)
    for k in range(NCORES):
        o = np.asarray(results[k]["out"]).astype(np.float32) / 255.0
        o = o.reshape(3, NSB, ROWS, CB)
        img[:, k * ROWS:(k + 1) * ROWS, :] = o.transpose(0, 2, 1, 3).reshape(
            3, ROWS, W)
    return img[None]


def kernel(**inputs):
    inputs = {k: np.asarray(v, np.float32) for k, v in inputs.items()}
    in_maps, nlc, nhc, actL, actH = _host_inputs(**inputs)
    key = (nlc, nhc, actL, actH)
    if key not in _CACHE:
        _CACHE[key] = _build_program(nlc, nhc, actL, actH)
    nc = _CACHE[key]
    res = run_bass_kernel_spmd(nc, in_maps, list(range(NCORES)))
    return _assemble(res.results).astype(np.float32)


if __name__ == "__main__":
    import reference
    ins = {k: np.asarray(v) for k, v in reference.setup_inputs().items()}
    out = kernel(**ins)
    ref = np.asarray(reference.reference(**reference.setup_inputs()))
    rel = np.linalg.norm(out - ref) / np.linalg.norm(ref)
    print("Relative error:", rel)
